# revision 20
# baseline (speedup 1.0000x reference)
"""Trainium2 Bass kernel for nn_GCLSTM (gnn_message_passing).

Architecture notes (all derived from the reference computation):
  * Every LSTMCell runs with zero initial state, so there is no recurrence:
    h = sigmoid(o) * tanh(sigmoid(i) * tanh(g)) per (batch, time) sample;
    the forget gate is unused.
  * fuse2 @ Wout collapses to a single 600-vector W2, so the final head is
    predict = sum_c w_c * (h_c . W2top) + htarget . W2bot + beta.  The
    per-channel scalar r_c = h_c . W2top rides the score matmul as a second
    output column; fusion (300 dims) is never materialized.
  * The reference's raw [T,12,B] -> [T,B,12] reshape of the softmaxed
    attention weights mixes batch elements within a timestep, so we shard
    the 8 cores over T (8 timesteps per core) - pure data parallelism with
    the scramble kept core-local.

Layout: features on partitions, samples (t_local*128 + b) on the free dim.
Gate matmuls use a zero-padded block-diagonal lhsT over the 96-row
(feature x channel) transposed input so all 12 channels share one rhs.
Gate M-order is [i, o, g], each padded to 384 columns so that every matmul
M-tile is a full 128 partitions (zero columns produce benign zeros in the
junk lanes, which are excluded from all downstream matmul reads).
"""

import os
import sys

import numpy as np

for _p in ("/opt/trn_rl_repo",):
    if os.path.isdir(_p) and _p not in sys.path:
        sys.path.insert(0, _p)

import concourse.bacc as bacc
import concourse.bass as bass
import concourse.mybir as mybir
from concourse.bass_utils import run_bass_kernel_spmd
from concourse.tile import TileContext

F32 = mybir.dt.float32
F32R = mybir.dt.float32r
BF16 = mybir.dt.bfloat16
AF = mybir.ActivationFunctionType

H = 300
B, T = 128, 64
NCORES = 8
TL = T // NCORES      # timesteps per core
N = B * TL            # samples per core
CK = 512              # free-dim chunk for the heavy matmuls
NCK = N // CK
C = 12                # neighbor channels

# att1 K-tile partition sizes: h tiles (128,128,44), htarget tiles (128,128,65)
# (partition 64 of the last htarget tile is the ones row carrying ba1; engine
# APs must start 32-aligned, and junk lanes 44..63 are exact zeros).
A1_KP = (128, 128, 44, 128, 128, 65)
RT_KP = (128, 128, 65)


def _gates_h(nc, gpool, wpool, lhsT, kp, rhs, h_out):
    """Gates matmul + LSTM zero-state nonlinearity for one (channel, chunk).

    lhsT: [kp, 1152] (3 waves x 384 cols, order i,o,g, zero-padded),
    rhs:  [kp, CK], h_out: [128, 1536] (valid: [0:1024] + rows 0:44 of
    [1024:1536]).
    """
    waves = []
    for w in range(3):
        gw = gpool.tile([128, 3 * CK], F32, tag="gw", name=f"gw{w}")
        for j in range(3):
            nc.tensor.matmul(
                out=gw[:, j * CK:(j + 1) * CK],
                lhsT=lhsT[:, w * 384 + j * 128: w * 384 + (j + 1) * 128],
                rhs=rhs,
                start=True,
                stop=True,
            )
        waves.append(gw)
    sgi = wpool.tile([128, 3 * CK], BF16, tag="sgi", name="sgi")
    nc.scalar.activation(sgi[:, :], waves[0][:, :], AF.Sigmoid)
    sgo = wpool.tile([128, 3 * CK], BF16, tag="sgo", name="sgo")
    nc.scalar.activation(sgo[:, :], waves[1][:, :], AF.Sigmoid)
    tg = wpool.tile([128, 3 * CK], BF16, tag="tg", name="tg")
    nc.scalar.activation(tg[:, :], waves[2][:, :], AF.Tanh)
    cp = wpool.tile([128, 3 * CK], BF16, tag="cp", name="cp")
    nc.vector.tensor_mul(cp[:, :], sgi[:, :], tg[:, :])
    tcn = wpool.tile([128, 3 * CK], BF16, tag="tcn", name="tcn")
    nc.scalar.activation(tcn[:, :], cp[:, :], AF.Tanh)
    nc.vector.tensor_mul(h_out[:, :], sgo[:, :], tcn[:, :])


def _h_ktiles(h, ht):
    """The six att1 K-tile rhs APs: h tiles then htarget tiles."""
    return [
        h[:, 0:CK],
        h[:, CK:2 * CK],
        h[0:44, 2 * CK:3 * CK],
        ht[:, 0:CK],
        ht[:, CK:2 * CK],
        ht[0:65, 2 * CK:3 * CK],
    ]


def _build():
    nc = bacc.Bacc("TRN2", target_bir_lowering=False, debug=False)

    li = nc.declare_dram_parameter("li", [B, TL, 20, 12], F32, isOutput=False)
    exb = nc.declare_dram_parameter("exb", [6, N], BF16, isOutput=False)
    lbt = nc.declare_dram_parameter("lbt", [1, N], F32, isOutput=False)
    wgz = nc.declare_dram_parameter("wgz", [97, C * 1152], BF16, isOutput=False)
    wtz = nc.declare_dram_parameter("wtz", [6, 1152], BF16, isOutput=False)
    a1w = nc.declare_dram_parameter("a1w", [128, 6 * 225], BF16, isOutput=False)
    sca = nc.declare_dram_parameter("sca", [128, 4], BF16, isOutput=False)
    scz = nc.declare_dram_parameter("scz", [97, C], BF16, isOutput=False)
    rtw = nc.declare_dram_parameter("rtw", [128, 3], BF16, isOutput=False)
    idn = nc.declare_dram_parameter("idn", [128, 128], F32, isOutput=False)
    on12 = nc.declare_dram_parameter("on12", [12, 1], BF16, isOutput=False)
    o112 = nc.declare_dram_parameter("o112", [1, 12], BF16, isOutput=False)
    outp = nc.declare_dram_parameter("outp", [1, N], F32, isOutput=True)
    outl = nc.declare_dram_parameter("outl", [1, N], F32, isOutput=True)

    with TileContext(nc) as tc:
        with (
            tc.sbuf_pool(name="cpool", bufs=1) as cpool,
            tc.sbuf_pool(name="wpool", bufs=2) as wpool,
            tc.sbuf_pool(name="hpool", bufs=3) as hpool,
            tc.psum_pool(name="gpool", bufs=2) as gpool,
            tc.psum_pool(name="apool", bufs=1) as apool,
        ):
            # ---- constants / weights to SBUF ----
            idn_sb = cpool.tile_from(idn[:, :], name="idn_sb")
            # gate weights: one DMA per channel so each matmul slice has a
            # single writer (avoids PE wait-slot overflow from queue fanout)
            wg_sb = cpool.tile([97, C * 1152], BF16, name="wg_sb")
            for c in range(C):
                nc.sync.dma_start(
                    out=wg_sb[:, c * 1152:(c + 1) * 1152],
                    in_=wgz[:, c * 1152:(c + 1) * 1152],
                )
            wt_sb = cpool.tile_from(wtz[:, :], name="wt_sb")
            a1_sb = cpool.tile([128, 6 * 225], BF16, name="a1_sb")
            for j in range(6):
                nc.sync.dma_start(
                    out=a1_sb[:, j * 225:(j + 1) * 225],
                    in_=a1w[:, j * 225:(j + 1) * 225],
                )
            sca_sb = cpool.tile_from(sca[:, :], name="sca_sb")
            scz_sb = cpool.tile_from(scz[:, :], name="scz_sb")
            rtw_sb = cpool.tile_from(rtw[:, :], name="rtw_sb")
            on12_sb = cpool.tile_from(on12[:, :], name="on12_sb")
            o112_sb = cpool.tile_from(o112[:, :], name="o112_sb")

            # ---- labels passthrough (host-transposed) ----
            nc.sync.dma_start(out=outl[:, :], in_=lbt[:, :])

            # ---- neighbor input S: load contiguous, transpose on PE ----
            # sraw[b, t*96 + f*12 + c] = local_inputs[b, t, f, c] (f < 8)
            sraw = cpool.tile([128, TL * 96], F32, name="sraw")
            nc.sync.dma_start(out=sraw[:, :], in_=li[:, :, 0:8, :])
            # str_[f*12 + c, t*128 + b]; row 96 = ones (bias lane)
            str_ = cpool.tile([97, N], BF16, name="str_")
            nc.vector.memset(str_[96:97, :], 1.0)
            for t in range(TL):
                tp = gpool.tile([96, 128], F32, tag="gw", name="tp")
                nc.tensor.transpose(
                    out=tp[:, :], in_=sraw[:, t * 96:(t + 1) * 96],
                    identity=idn_sb[:, :],
                )
                nc.vector.tensor_copy(str_[0:96, t * 128:(t + 1) * 128], tp[:, :])

            # ---- target input (host-packed, transposed, ones row baked in) ----
            tsr = cpool.tile_from(exb[:, :], name="tsr")

            # ---- score / r accumulation targets: [12, N] partition-c layout ----
            sct = cpool.tile([12, N], F32, name="sct")
            r12t = cpool.tile([12, N], F32, name="r12t")
            rt_sb = cpool.tile([1, N], F32, name="rt_sb")

            # ---- target LSTM h per chunk ----
            hts = []
            for k in range(NCK):
                ht = cpool.tile([128, 3 * CK], BF16, tag=f"ht{k}", name=f"ht{k}")
                _gates_h(nc, gpool, wpool, wt_sb[0:6, :], 6,
                         tsr[0:6, k * CK:(k + 1) * CK], ht)
                # ones lane for the ba1 bias row (partition 64 of the rem tile)
                nc.vector.memset(ht[64:65, 2 * CK:3 * CK], 1.0)
                hts.append(ht)

                # r_t = htarget . W2bot + beta (rides the ones lane)
                rtp = apool.tile([1, CK], F32, tag="vp", name="rtp")
                ht_tiles = _h_ktiles(ht, ht)[3:]
                for j in range(3):
                    nc.tensor.matmul(
                        out=rtp[:, :],
                        lhsT=rtw_sb[0:RT_KP[j], j:j + 1],
                        rhs=ht_tiles[j],
                        start=(j == 0),
                        stop=(j == 2),
                    )
                nc.vector.tensor_copy(rt_sb[0:1, k * CK:(k + 1) * CK], rtp[:, :])

            # ---- main loop over (chunk, channel) ----
            for k in range(NCK):
                rhs_chunk = str_[:, k * CK:(k + 1) * CK]
                for c in range(C):
                    h = hpool.tile([128, 3 * CK], BF16, tag="h", name="h")
                    _gates_h(nc, gpool, wpool,
                             wg_sb[:, c * 1152:(c + 1) * 1152], 97,
                             rhs_chunk, h)

                    ktiles = _h_ktiles(h, hts[k])

                    # att1: a_pre = [h; htarget; 1] @ [A1bot; A1top; ba1]
                    vp = apool.tile([128, 2 * CK], F32, tag="vp", name="vp")
                    for j in range(6):
                        kpj = A1_KP[j]
                        for m, (mo, mw) in enumerate(((0, 128), (128, 97))):
                            nc.tensor.matmul(
                                out=vp[0:mw, m * CK:m * CK + CK],
                                lhsT=a1_sb[0:kpj, j * 225 + mo: j * 225 + mo + mw],
                                rhs=ktiles[j],
                                start=(j == 0),
                                stop=(j == 5),
                            )
                    a = wpool.tile([128, 2 * CK], BF16, tag="a", name="a")
                    nc.vector.tensor_scalar_max(a[0:128, 0:CK], vp[0:128, 0:CK], 0.0)
                    nc.vector.tensor_scalar_max(
                        a[0:72, CK:2 * CK], vp[0:72, CK:2 * CK], 0.0
                    )

                    # score (r_c was computed in att1 at partition 96)
                    sp = gpool.tile([1, CK], F32, tag="gw", name="sp")
                    sc_tiles = [
                        (sca_sb[0:128, 0:1], a[0:128, 0:CK]),
                        (sca_sb[0:72, 2:3], a[0:72, CK:2 * CK]),
                        (scz_sb[0:97, c:c + 1], rhs_chunk),
                    ]
                    for j, (lt, rt_) in enumerate(sc_tiles):
                        nc.tensor.matmul(
                            out=sp[:, :],
                            lhsT=lt,
                            rhs=rt_,
                            start=(j == 0),
                            stop=(j == 2),
                        )
                    scx = wpool.tile([1, CK], F32, tag="scx", name="scx")
                    nc.vector.tensor_scalar_max(scx[:, :], sp[0:1, :], 0.0)
                    nc.sync.dma_start(
                        out=sct[c:c + 1, k * CK:(k + 1) * CK], in_=scx[:, :]
                    )
                    rcx = wpool.tile([1, CK], F32, tag="rcx", name="rcx")
                    nc.vector.tensor_copy(rcx[:, :], vp[96:97, CK:2 * CK])
                    nc.sync.dma_start(
                        out=r12t[c:c + 1, k * CK:(k + 1) * CK], in_=rcx[:, :]
                    )

            # ---- softmax over channels (per t) + scramble + head ----
            e_sb = cpool.tile([12, N], BF16, name="e_sb")
            nc.scalar.activation(e_sb[:, :], sct[:, :], AF.Exp)
            dp = apool.tile([1, N], F32, tag="vp", name="dp")
            for q in range(NCK):
                nc.tensor.matmul(
                    out=dp[:, q * CK:(q + 1) * CK], lhsT=on12_sb[:, :],
                    rhs=e_sb[:, q * CK:(q + 1) * CK], start=True, stop=True,
                )
            rc = cpool.tile([1, N], BF16, name="rc")
            with nc.allow_low_precision("softmax denom fits bf16 here"):
                nc.vector.reciprocal(rc[:, :], dp[:, :])
            rp = apool.tile([12, N], F32, tag="vp", name="rp")
            for q in range(NCK):
                nc.tensor.matmul(
                    out=rp[:, q * CK:(q + 1) * CK], lhsT=o112_sb[:, :],
                    rhs=rc[:, q * CK:(q + 1) * CK], start=True, stop=True,
                )
            wn = cpool.tile([12, N], F32, name="wn")
            nc.vector.tensor_mul(wn[:, :], e_sb[:, :], rp[:, :])

            # scramble: w_used[b, c] = flat[b*12 + c], flat = row-major [12, B]
            wu = cpool.tile([12, N], F32, name="wu")
            wnd = nc.dram_tensor("wnd", [TL, 12 * B], F32)
            for t in range(TL):
                nc.sync.dma_start(
                    out=wnd[t, :], in_=wn[:, t * B:(t + 1) * B]
                )
                scr = wnd[t, :].rearrange("(b c) -> c b", c=12)
                with nc.allow_non_contiguous_dma("softmax weight scramble"):
                    nc.sync.dma_start(
                        out=wu[:, t * B:(t + 1) * B], in_=scr,
                    )

            pd = cpool.tile([12, N], BF16, name="pd")
            nc.vector.tensor_mul(pd[:, :], wu[:, :], r12t[:, :])
            pp = apool.tile([1, N], F32, tag="vp", name="pp")
            for q in range(NCK):
                nc.tensor.matmul(
                    out=pp[:, q * CK:(q + 1) * CK], lhsT=on12_sb[:, :],
                    rhs=pd[:, q * CK:(q + 1) * CK], start=True, stop=True,
                )
            outs = cpool.tile([1, N], F32, name="outs")
            nc.vector.scalar_tensor_tensor(
                out=outs[:, :], in0=pp[:, :], scalar=1.0, in1=rt_sb[:, :],
                op0=mybir.AluOpType.mult, op1=mybir.AluOpType.add,
            )
            nc.sync.dma_start(out=outp[:, :], in_=outs[:, :])

    if not nc.is_finalized():
        nc.finalize()
    return nc


def _prep_weights(W_ih, b_ih, b_hh, Wt_ih, bt_ih, bt_hh,
                  Att1, ba1, Att2, ba2, fuse2, biasf2, Wout, biasout):
    """Host-side packing of all stationary operands (fp32 numpy)."""
    f32 = np.float32
    # gate row ranges in PyTorch order i,f,g,o; kernel M-order is [i, o, g]
    gsel = [np.arange(0, 300), np.arange(900, 1200), np.arange(600, 900)]
    bias_n = (b_ih + b_hh).astype(f32)            # [12, 1200]

    wgz = np.zeros((97, C * 1152), dtype=f32)
    for c in range(C):
        blk = np.zeros((97, 1152), dtype=f32)
        for w, rows in enumerate(gsel):
            Wsub = W_ih[c][rows, :]               # [300, 8]
            for f in range(8):
                blk[f * 12 + c, w * 384: w * 384 + 300] = Wsub[:, f]
            blk[96, w * 384: w * 384 + 300] = bias_n[c][rows]
        wgz[:, c * 1152:(c + 1) * 1152] = blk

    wtz = np.zeros((6, 1152), dtype=f32)
    bias_t = (bt_ih + bt_hh).astype(f32)
    for w, rows in enumerate(gsel):
        Wsub = Wt_ih[rows, :]                     # [300, 5]
        for f in range(5):
            wtz[f, w * 384: w * 384 + 300] = Wsub[:, f]
        wtz[5, w * 384: w * 384 + 300] = bias_t[rows]

    A1top = Att1[0:300, :]                        # pairs htarget
    A1bot = Att1[300:600, :]                      # pairs h
    W2 = (fuse2 @ Wout).astype(f32)[:, 0]         # [600]
    W2top, W2bot = W2[0:300], W2[300:600]
    # per K-tile: [Mtile0 (200 a-cols split 128|72 at offsets 0 and 128),
    #              zeros 200..223, r-col 224 = W2top (h tiles only)]
    a1w = np.zeros((128, 6 * 225), dtype=f32)
    ksrc = [A1bot[0:128], A1bot[128:256], A1bot[256:300],
            A1top[0:128], A1top[128:256], A1top[256:300]]
    rsrc = [W2top[0:128], W2top[128:256], W2top[256:300], None, None, None]
    for j, src in enumerate(ksrc):
        kp = src.shape[0]
        a1w[0:kp, j * 225: j * 225 + 128] = src[:, 0:128]
        a1w[0:kp, j * 225 + 128: j * 225 + 200] = src[:, 128:200]
        if rsrc[j] is not None:
            a1w[0:kp, j * 225 + 224] = rsrc[j]
    a1w[64, 5 * 225: 5 * 225 + 128] = ba1[0:128]  # ones-lane bias row
    a1w[64, 5 * 225 + 128: 5 * 225 + 200] = ba1[128:200]

    beta = float(biasf2 @ Wout[:, 0] + biasout[0])

    sca = np.zeros((128, 4), dtype=f32)
    sca[0:128, 0] = Att2[0:128, 0]
    sca[0:72, 2] = Att2[128:200, 0]

    scz = np.zeros((97, C), dtype=f32)
    for c in range(C):
        scz[84 + c, c] = Att2[200, 0]
        scz[72 + c, c] = Att2[201, 0]
        scz[96, c] = ba2[0]

    rtw = np.zeros((128, 3), dtype=f32)
    rtw[0:128, 0] = W2bot[0:128]
    rtw[0:128, 1] = W2bot[128:256]
    rtw[0:44, 2] = W2bot[256:300]
    rtw[64, 2] = beta

    try:
        import ml_dtypes
        bf16 = ml_dtypes.bfloat16
    except ImportError:  # pragma: no cover
        import jax.numpy as jnp
        bf16 = jnp.bfloat16
    return {
        "wgz": wgz.astype(bf16), "wtz": wtz.astype(bf16),
        "a1w": a1w.astype(bf16), "sca": sca.astype(bf16),
        "scz": scz.astype(bf16),
        "rtw": rtw.astype(bf16),
        "idn": np.eye(128, dtype=f32),
        "on12": np.ones((12, 1), dtype=bf16),
        "o112": np.ones((1, 12), dtype=bf16),
    }


_CACHE = {}


def kernel(**inputs):
    inp = {k: np.ascontiguousarray(np.asarray(v, dtype=np.float32))
           for k, v in inputs.items()}

    if "nc" not in _CACHE:
        _CACHE["nc"] = _build()
    nc = _CACHE["nc"]

    wmap = _prep_weights(
        inp["W_ih"], inp["b_ih"], inp["b_hh"], inp["Wt_ih"], inp["bt_ih"],
        inp["bt_hh"], inp["Att1"], inp["ba1"], inp["Att2"], inp["ba2"],
        inp["fuse2"], inp["biasf2"], inp["Wout"], inp["biasout"],
    )

    li, lbl, exr = inp["local_inputs"], inp["labels"], inp["extras"]
    in_maps = []
    for k in range(NCORES):
        ts = slice(k * TL, (k + 1) * TL)
        m = dict(wmap)
        m["li"] = np.ascontiguousarray(li[:, ts])
        ex_t = np.ones((6, N), dtype=np.float32)
        # [5, t, b] <- extras[b, t, f, 0]
        ex_t[0:5] = exr[:, ts, 0:5, 0].transpose(2, 1, 0).reshape(5, N)
        m["exb"] = ex_t.astype(wmap["wgz"].dtype)
        m["lbt"] = np.ascontiguousarray(
            lbl[:, ts, 0, 0].T.reshape(1, N).astype(np.float32))
        in_maps.append(m)

    res = run_bass_kernel_spmd(nc, in_maps, list(range(NCORES))).results

    predicts = np.concatenate(
        [res[k]["outp"].reshape(TL, B) for k in range(NCORES)], axis=0
    ).reshape(T, B, 1)
    labels_out = np.concatenate(
        [res[k]["outl"].reshape(TL, B) for k in range(NCORES)], axis=0
    ).reshape(T, B, 1)
    return predicts, labels_out


# revision 33
# speedup vs baseline: 2.2585x; 2.2585x over previous
"""Trainium2 Bass kernel for nn_GCLSTM (gnn_message_passing).

Architecture notes (all derived from the reference computation):
  * Every LSTMCell runs with zero initial state, so there is no recurrence:
    h = sigmoid(o) * tanh(sigmoid(i) * tanh(g)) per (batch, time) sample;
    the forget gate is unused.
  * fuse2 @ Wout collapses to a single 600-vector W2, so the final head is
    predict = sum_c w_c * (h_c . W2top) + htarget . W2bot + beta.  The
    per-channel scalar r_c = h_c . W2top rides the score matmul as a second
    output column; fusion (300 dims) is never materialized.
  * The reference's raw [T,12,B] -> [T,B,12] reshape of the softmaxed
    attention weights mixes batch elements within a timestep, so we shard
    the 8 cores over T (8 timesteps per core) - pure data parallelism with
    the scramble kept core-local.

Layout: features on partitions, samples (t_local*128 + b) on the free dim.
Gate matmuls use a zero-padded block-diagonal lhsT over the 96-row
(feature x channel) transposed input so all 12 channels share one rhs.
Gate M-order is [i, o, g], each padded to 384 columns so that every matmul
M-tile is a full 128 partitions (zero columns produce benign zeros in the
junk lanes, which are excluded from all downstream matmul reads).
"""

import os
import sys

import numpy as np

for _p in ("/opt/trn_rl_repo",):
    if os.path.isdir(_p) and _p not in sys.path:
        sys.path.insert(0, _p)

import concourse.bacc as bacc
import concourse.bass as bass
import concourse.mybir as mybir
from concourse.bass_utils import run_bass_kernel_spmd
from concourse.tile import TileContext

F32 = mybir.dt.float32
F32R = mybir.dt.float32r
BF16 = mybir.dt.bfloat16
AF = mybir.ActivationFunctionType

H = 300
B, T = 128, 64
NCORES = 8
TL = T // NCORES      # timesteps per core
N = B * TL            # samples per core
CK = 512              # free-dim chunk for the heavy matmuls
NCK = N // CK
C = 12                # neighbor channels

# att1 K-tile partition sizes: h tiles (128,128,44), htarget tiles (128,128,65)
# (partition 64 of the last htarget tile is the ones row carrying ba1; engine
# APs must start 32-aligned, and junk lanes 44..63 are exact zeros).
A1_KP = (128, 128, 44, 128, 128, 65)
RT_KP = (128, 128, 65)


def _gates_h(nc, gpool, wpool, lhsT, kp, rhs, h_out):
    """Gates matmul + LSTM zero-state nonlinearity for one (channel, chunk).

    lhsT: [kp, 1152] = 9 column-blocks of 128 in order
    [i0 i1 i44z | o0 o1 o44z | g0 g1 g44z]; processed as 5 PSUM waves of
    <=2 blocks: (i0,i1) (i44z,o0) (o1,o44z) -> sigmoid, (g0,g1) (g44z) ->
    tanh.  rhs: [kp, CK].  h_out: [128, 3*CK] bf16 (valid: blocks 0,1 and
    rows 0:44 of block 2).
    """
    sg = wpool.tile([128, 6 * CK], BF16, tag="sg", name="sg")
    tg = wpool.tile([128, 3 * CK], BF16, tag="tg", name="tg")
    cp = wpool.tile([128, 3 * CK], BF16, tag="cp", name="cp")

    def wave(w):
        nblk = 2 if w < 4 else 1
        gw = gpool.tile([128, 2 * CK], F32, tag="gw", name=f"gw{w}")
        for j in range(nblk):
            nc.tensor.matmul(
                out=gw[:, j * CK:(j + 1) * CK],
                lhsT=lhsT[:, w * 256 + j * 128: w * 256 + (j + 1) * 128],
                rhs=rhs,
                start=True,
                stop=True,
            )
        if w < 3:
            nc.scalar.activation(
                sg[:, w * 2 * CK:(w * 2 + nblk) * CK],
                gw[:, 0:nblk * CK], AF.Sigmoid,
            )
        else:
            off = (w - 3) * 2 * CK
            nc.scalar.activation(
                tg[:, off:off + nblk * CK], gw[:, 0:nblk * CK], AF.Tanh,
            )

    for w in range(5):
        wave(w)
    nc.vector.tensor_mul(cp[:, :], sg[:, 0:3 * CK], tg[:, :])
    tcn = wpool.tile([128, 3 * CK], BF16, tag="tcn", name="tcn")
    nc.scalar.activation(tcn[:, :], cp[:, :], AF.Tanh)
    nc.vector.tensor_mul(h_out[:, :], sg[:, 3 * CK:6 * CK], tcn[:, :])


def _h_ktiles(h, ht):
    """The six att1 K-tile rhs APs: h tiles then htarget tiles."""
    return [
        h[:, 0:CK],
        h[:, CK:2 * CK],
        h[0:44, 2 * CK:3 * CK],
        ht[:, 0:CK],
        ht[:, CK:2 * CK],
        ht[0:65, 2 * CK:3 * CK],
    ]


def _build():
    nc = bacc.Bacc("TRN2", target_bir_lowering=False, debug=False)

    li = nc.declare_dram_parameter("li", [B, TL, 20, 12], F32, isOutput=False)
    exb = nc.declare_dram_parameter("exb", [6, N], BF16, isOutput=False)
    lbt = nc.declare_dram_parameter("lbt", [1, N], F32, isOutput=False)
    wgz = nc.declare_dram_parameter("wgz", [97, C * 1152], BF16, isOutput=False)
    wtz = nc.declare_dram_parameter("wtz", [6, 1152], BF16, isOutput=False)
    a1w = nc.declare_dram_parameter("a1w", [128, 6 * 225], BF16, isOutput=False)
    sca = nc.declare_dram_parameter("sca", [128, 4], BF16, isOutput=False)
    scz = nc.declare_dram_parameter("scz", [97, C], BF16, isOutput=False)
    rtw = nc.declare_dram_parameter("rtw", [128, 3], BF16, isOutput=False)
    idn = nc.declare_dram_parameter("idn", [128, 128], F32, isOutput=False)
    on12 = nc.declare_dram_parameter("on12", [12, 1], BF16, isOutput=False)
    o112 = nc.declare_dram_parameter("o112", [1, 12], BF16, isOutput=False)
    outp = nc.declare_dram_parameter("outp", [1, N], F32, isOutput=True)
    outl = nc.declare_dram_parameter("outl", [1, N], F32, isOutput=True)

    with TileContext(nc) as tc:
        with (
            tc.sbuf_pool(name="cpool", bufs=1) as cpool,
            tc.sbuf_pool(name="wpool", bufs=3) as wpool,
            tc.sbuf_pool(name="hpool", bufs=3) as hpool,
            tc.psum_pool(name="gpool", bufs=2) as gpool,
            tc.psum_pool(name="apool", bufs=2) as apool,
        ):
            # ---- inputs first: they gate the whole pipeline ----
            idn_sb = cpool.tile_from(idn[:, :], name="idn_sb")
            # sraw[b, t*96 + f*12 + c] = local_inputs[b, t, f, c] (f < 8)
            sraw = cpool.tile([128, TL * 96], F32, name="sraw")
            nc.sync.dma_start(out=sraw[:, :], in_=li[:, :, 0:8, :])
            # target input (host-packed, transposed, ones row baked in)
            tsr = cpool.tile_from(exb[:, :], name="tsr")
            wt_sb = cpool.tile_from(wtz[:, :], name="wt_sb")

            # str_[f*12 + c, t*128 + b]; row 96 = ones (bias lane)
            str_ = cpool.tile([97, N], BF16, name="str_")
            nc.vector.memset(str_[96:97, :], 1.0)
            for t in range(TL):
                tp = apool.tile([96, 128], F32, tag="vp", name="tp")
                nc.tensor.transpose(
                    out=tp[:, :], in_=sraw[:, t * 96:(t + 1) * 96],
                    identity=idn_sb[:, :],
                )
                nc.vector.tensor_copy(str_[0:96, t * 128:(t + 1) * 128], tp[:, :])

            # gate weights: one DMA per channel so each matmul slice has a
            # single writer (avoids PE wait-slot overflow from queue fanout)
            wg_sb = cpool.tile([97, C * 1152], BF16, name="wg_sb")
            for c in range(C):
                nc.sync.dma_start(
                    out=wg_sb[:, c * 1152:(c + 1) * 1152],
                    in_=wgz[:, c * 1152:(c + 1) * 1152],
                )
            a1_sb = cpool.tile([128, 6 * 225], BF16, name="a1_sb")
            for j in range(6):
                nc.sync.dma_start(
                    out=a1_sb[:, j * 225:(j + 1) * 225],
                    in_=a1w[:, j * 225:(j + 1) * 225],
                )
            sca_sb = cpool.tile_from(sca[:, :], name="sca_sb")
            scz_sb = cpool.tile_from(scz[:, :], name="scz_sb")
            rtw_sb = cpool.tile_from(rtw[:, :], name="rtw_sb")
            on12_sb = cpool.tile_from(on12[:, :], name="on12_sb")
            o112_sb = cpool.tile_from(o112[:, :], name="o112_sb")

            # ---- labels passthrough (host-transposed) ----
            nc.sync.dma_start(out=outl[:, :], in_=lbt[:, :])

            # ---- r_t target ----
            rt_sb = cpool.tile([1, N], F32, name="rt_sb")
            outs = cpool.tile([1, N], F32, name="outs")
            wnd = nc.dram_tensor("wnd", [TL, 12 * B], BF16)

            hts = {}

            def make_ht(k):
                ht = cpool.tile([128, 3 * CK], BF16, tag=f"ht{k}", name=f"ht{k}")
                _gates_h(nc, gpool, wpool, wt_sb[0:6, :], 6,
                         tsr[0:6, k * CK:(k + 1) * CK], ht)
                # ones lane for the ba1 bias row (partition 64 of the rem tile)
                nc.vector.memset(ht[64:65, 2 * CK:3 * CK], 1.0)
                hts[k] = ht

                # r_t = htarget . W2bot + beta (rides the ones lane)
                rtp = apool.tile([1, CK], F32, tag="vp", name="rtp")
                ht_tiles = _h_ktiles(ht, ht)[3:]
                for j in range(3):
                    nc.tensor.matmul(
                        out=rtp[:, :],
                        lhsT=rtw_sb[0:RT_KP[j], j:j + 1],
                        rhs=ht_tiles[j],
                        start=(j == 0),
                        stop=(j == 2),
                    )
                nc.vector.tensor_copy(rt_sb[0:1, k * CK:(k + 1) * CK], rtp[:, :])

            # ---- main loop over chunks: channels, then softmax + head ----
            for k in range(NCK):
                rhs_chunk = str_[:, k * CK:(k + 1) * CK]
                scst = wpool.tile([1, C * CK], BF16, tag="scst", name="scst")
                rcst = wpool.tile([1, C * CK], BF16, tag="rcst", name="rcst")
                def tail_att(c, h):
                    ktiles = _h_ktiles(h, hts[k])

                    # att1: a_pre = [h; htarget; 1] @ [A1bot; A1top; ba1]
                    vp = apool.tile([128, 2 * CK], F32, tag="vp", name="vp")
                    for j in range(6):
                        kpj = A1_KP[j]
                        for m, (mo, mw) in enumerate(((0, 128), (128, 97))):
                            nc.tensor.matmul(
                                out=vp[0:mw, m * CK:m * CK + CK],
                                lhsT=a1_sb[0:kpj, j * 225 + mo: j * 225 + mo + mw],
                                rhs=ktiles[j],
                                start=(j == 0),
                                stop=(j == 5),
                            )
                    a = wpool.tile([128, 2 * CK], BF16, tag="a", name="a")
                    nc.vector.tensor_scalar_max(a[0:128, 0:CK], vp[0:128, 0:CK], 0.0)
                    nc.vector.tensor_scalar_max(
                        a[0:72, CK:2 * CK], vp[0:72, CK:2 * CK], 0.0
                    )
                    # r_c sits at partition 96 of the second M-tile
                    nc.vector.tensor_copy(
                        rcst[0:1, c * CK:(c + 1) * CK], vp[96:97, CK:2 * CK]
                    )

                    # score (relu into the staging strip)
                    sp = apool.tile([1, CK], F32, tag="vp", name="sp")
                    sc_tiles = [
                        (sca_sb[0:128, 0:1], a[0:128, 0:CK]),
                        (sca_sb[0:72, 2:3], a[0:72, CK:2 * CK]),
                        (scz_sb[0:97, c:c + 1], rhs_chunk),
                    ]
                    for j, (lt, rt_) in enumerate(sc_tiles):
                        nc.tensor.matmul(
                            out=sp[:, :],
                            lhsT=lt,
                            rhs=rt_,
                            start=(j == 0),
                            stop=(j == 2),
                        )
                    nc.vector.tensor_scalar_max(
                        scst[0:1, c * CK:(c + 1) * CK], sp[0:1, :], 0.0
                    )

                # software pipeline: emit gates(c+1) before att1/score(c) so
                # the activation engine's next sigma wave is never stuck
                # behind att1 matmuls on PE
                h_prev = None
                c_prev = -1
                for c in range(C):
                    h = hpool.tile([128, 3 * CK], BF16, tag="h", name="h")
                    _gates_h(nc, gpool, wpool,
                             wg_sb[:, c * 1152:(c + 1) * 1152], 97,
                             rhs_chunk, h)
                    if c == 0:
                        make_ht(k)
                    if h_prev is not None:
                        tail_att(c_prev, h_prev)
                    h_prev, c_prev = h, c
                tail_att(c_prev, h_prev)

                # scatter staging strips to [12, CK] partition-c layout
                sctk = wpool.tile([12, CK], BF16, tag="sctk", name="sctk")
                nc.sync.dma_start(out=sctk[:, :], in_=scst[0:1, :])
                rctk = wpool.tile([12, CK], BF16, tag="rctk", name="rctk")
                nc.sync.dma_start(out=rctk[:, :], in_=rcst[0:1, :])

                # softmax over channels (no max-sub: scores are relu'd, small)
                ek = wpool.tile([12, CK], BF16, tag="ek", name="ek")
                nc.scalar.activation(ek[:, :], sctk[:, :], AF.Exp)
                dpk = apool.tile([1, CK], F32, tag="vp", name="dpk")
                nc.tensor.matmul(
                    out=dpk[:, :], lhsT=on12_sb[:, :], rhs=ek[:, :],
                    start=True, stop=True,
                )
                rck = wpool.tile([1, CK], BF16, tag="rck", name="rck")
                with nc.allow_low_precision("softmax denom fits bf16 here"):
                    nc.vector.reciprocal(rck[:, :], dpk[:, :])
                rpk = apool.tile([12, CK], F32, tag="vp", name="rpk")
                nc.tensor.matmul(
                    out=rpk[:, :], lhsT=o112_sb[:, :], rhs=rck[:, :],
                    start=True, stop=True,
                )
                wnk = wpool.tile([12, CK], BF16, tag="wnk", name="wnk")
                nc.vector.tensor_mul(wnk[:, :], ek[:, :], rpk[:, :])

                # scramble via DRAM bounce: w_used[b, c] = flat[b*12 + c]
                wuk = wpool.tile([12, CK], BF16, tag="wuk", name="wuk")
                for t4 in range(CK // B):
                    t = k * (CK // B) + t4
                    nc.sync.dma_start(
                        out=wnd[t, :], in_=wnk[:, t4 * B:(t4 + 1) * B]
                    )
                    scr = wnd[t, :].rearrange("(b c) -> c b", c=12)
                    with nc.allow_non_contiguous_dma("softmax weight scramble"):
                        nc.sync.dma_start(
                            out=wuk[:, t4 * B:(t4 + 1) * B], in_=scr,
                        )

                # predict = sum_c w_c * r_c + r_t
                pdk = wpool.tile([12, CK], BF16, tag="pdk", name="pdk")
                nc.vector.tensor_mul(pdk[:, :], wuk[:, :], rctk[:, :])
                ppk = apool.tile([1, CK], F32, tag="vp", name="ppk")
                nc.tensor.matmul(
                    out=ppk[:, :], lhsT=on12_sb[:, :], rhs=pdk[:, :],
                    start=True, stop=True,
                )
                nc.vector.scalar_tensor_tensor(
                    out=outs[0:1, k * CK:(k + 1) * CK], in0=ppk[:, :],
                    scalar=1.0, in1=rt_sb[0:1, k * CK:(k + 1) * CK],
                    op0=mybir.AluOpType.mult, op1=mybir.AluOpType.add,
                )
                nc.sync.dma_start(
                    out=outp[0:1, k * CK:(k + 1) * CK],
                    in_=outs[0:1, k * CK:(k + 1) * CK],
                )

    if not nc.is_finalized():
        nc.finalize()
    return nc


def _prep_weights(W_ih, b_ih, b_hh, Wt_ih, bt_ih, bt_hh,
                  Att1, ba1, Att2, ba2, fuse2, biasf2, Wout, biasout):
    """Host-side packing of all stationary operands (fp32 numpy)."""
    f32 = np.float32
    # gate row ranges in PyTorch order i,f,g,o; kernel M-order is [i, o, g]
    gsel = [np.arange(0, 300), np.arange(900, 1200), np.arange(600, 900)]
    bias_n = (b_ih + b_hh).astype(f32)            # [12, 1200]

    wgz = np.zeros((97, C * 1152), dtype=f32)
    for c in range(C):
        blk = np.zeros((97, 1152), dtype=f32)
        for w, rows in enumerate(gsel):
            Wsub = W_ih[c][rows, :]               # [300, 8]
            for f in range(8):
                blk[f * 12 + c, w * 384: w * 384 + 300] = Wsub[:, f]
            blk[96, w * 384: w * 384 + 300] = bias_n[c][rows]
        wgz[:, c * 1152:(c + 1) * 1152] = blk

    wtz = np.zeros((6, 1152), dtype=f32)
    bias_t = (bt_ih + bt_hh).astype(f32)
    for w, rows in enumerate(gsel):
        Wsub = Wt_ih[rows, :]                     # [300, 5]
        for f in range(5):
            wtz[f, w * 384: w * 384 + 300] = Wsub[:, f]
        wtz[5, w * 384: w * 384 + 300] = bias_t[rows]

    A1top = Att1[0:300, :]                        # pairs htarget
    A1bot = Att1[300:600, :]                      # pairs h
    W2 = (fuse2 @ Wout).astype(f32)[:, 0]         # [600]
    W2top, W2bot = W2[0:300], W2[300:600]
    # per K-tile: [Mtile0 (200 a-cols split 128|72 at offsets 0 and 128),
    #              zeros 200..223, r-col 224 = W2top (h tiles only)]
    a1w = np.zeros((128, 6 * 225), dtype=f32)
    ksrc = [A1bot[0:128], A1bot[128:256], A1bot[256:300],
            A1top[0:128], A1top[128:256], A1top[256:300]]
    rsrc = [W2top[0:128], W2top[128:256], W2top[256:300], None, None, None]
    for j, src in enumerate(ksrc):
        kp = src.shape[0]
        a1w[0:kp, j * 225: j * 225 + 128] = src[:, 0:128]
        a1w[0:kp, j * 225 + 128: j * 225 + 200] = src[:, 128:200]
        if rsrc[j] is not None:
            a1w[0:kp, j * 225 + 224] = rsrc[j]
    a1w[64, 5 * 225: 5 * 225 + 128] = ba1[0:128]  # ones-lane bias row
    a1w[64, 5 * 225 + 128: 5 * 225 + 200] = ba1[128:200]

    beta = float(biasf2 @ Wout[:, 0] + biasout[0])

    sca = np.zeros((128, 4), dtype=f32)
    sca[0:128, 0] = Att2[0:128, 0]
    sca[0:72, 2] = Att2[128:200, 0]

    scz = np.zeros((97, C), dtype=f32)
    for c in range(C):
        scz[84 + c, c] = Att2[200, 0]
        scz[72 + c, c] = Att2[201, 0]
        scz[96, c] = ba2[0]

    rtw = np.zeros((128, 3), dtype=f32)
    rtw[0:128, 0] = W2bot[0:128]
    rtw[0:128, 1] = W2bot[128:256]
    rtw[0:44, 2] = W2bot[256:300]
    rtw[64, 2] = beta

    try:
        import ml_dtypes
        bf16 = ml_dtypes.bfloat16
    except ImportError:  # pragma: no cover
        import jax.numpy as jnp
        bf16 = jnp.bfloat16
    return {
        "wgz": wgz.astype(bf16), "wtz": wtz.astype(bf16),
        "a1w": a1w.astype(bf16), "sca": sca.astype(bf16),
        "scz": scz.astype(bf16),
        "rtw": rtw.astype(bf16),
        "idn": np.eye(128, dtype=f32),
        "on12": np.ones((12, 1), dtype=bf16),
        "o112": np.ones((1, 12), dtype=bf16),
    }


_CACHE = {}


def kernel(**inputs):
    inp = {k: np.ascontiguousarray(np.asarray(v, dtype=np.float32))
           for k, v in inputs.items()}

    if "nc" not in _CACHE:
        _CACHE["nc"] = _build()
    nc = _CACHE["nc"]

    wmap = _prep_weights(
        inp["W_ih"], inp["b_ih"], inp["b_hh"], inp["Wt_ih"], inp["bt_ih"],
        inp["bt_hh"], inp["Att1"], inp["ba1"], inp["Att2"], inp["ba2"],
        inp["fuse2"], inp["biasf2"], inp["Wout"], inp["biasout"],
    )

    li, lbl, exr = inp["local_inputs"], inp["labels"], inp["extras"]
    in_maps = []
    for k in range(NCORES):
        ts = slice(k * TL, (k + 1) * TL)
        m = dict(wmap)
        m["li"] = np.ascontiguousarray(li[:, ts])
        ex_t = np.ones((6, N), dtype=np.float32)
        # [5, t, b] <- extras[b, t, f, 0]
        ex_t[0:5] = exr[:, ts, 0:5, 0].transpose(2, 1, 0).reshape(5, N)
        m["exb"] = ex_t.astype(wmap["wgz"].dtype)
        m["lbt"] = np.ascontiguousarray(
            lbl[:, ts, 0, 0].T.reshape(1, N).astype(np.float32))
        in_maps.append(m)

    res = run_bass_kernel_spmd(nc, in_maps, list(range(NCORES))).results

    predicts = np.concatenate(
        [res[k]["outp"].reshape(TL, B) for k in range(NCORES)], axis=0
    ).reshape(T, B, 1)
    labels_out = np.concatenate(
        [res[k]["outl"].reshape(TL, B) for k in range(NCORES)], axis=0
    ).reshape(T, B, 1)
    return predicts, labels_out
